# revision 21
# baseline (speedup 1.0000x reference)
"""Causal multihead self-attention with RoPE on 8 TRN2 NeuronCores.

Problem: B=2, S=2048, D=1024, H=16 heads, d_k=64, causal, RoPE theta=10000.

Sharding (Megatron-style): core c = 4*b + g handles batch b and the 4 heads
[4g, 4g+4): Wq/Wk/Wv column-parallel, Wo row-parallel; each core emits a
partial [S, D] fp16 output and the host sums the 4 partials per batch.

v3: software-pipelined across qtiles.  The attention phase (B) is
ACT-engine-bound (the softmax exp stream); the projection phase (A) and
output-projection phase (C) are PE-bound.  v3 interleaves the EMISSION of
A(t+1) and C(t-1) matmul units into B(t)'s chunk loop so the in-order PE
queue always has runnable work while ACT crunches exp — instead of the
phases running back-to-back with each engine idling in the other's phase.
Per-qtile activation tiles (qT/kT/yT/v_ext split by t) keep the interleaved
writes/reads on disjoint tiles so no false dependencies serialize the
pipeline.  Softmax 1/sums runs on DVE (reciprocal_approx_fast) instead of
ACT Ln+Exp; causal masking multiplies only the 128-col diagonal band (the
rest of a diagonal tile is fully masked -> skipped, or fully unmasked) and
runs on GpSimd; output partials are staged fp16.
"""
import sys

sys.path.insert(0, "/opt/trn_rl_repo")

import numpy as np

import concourse.bacc as bacc
import concourse.hw_specs as hw_specs
import concourse.tile as tile
from concourse import mybir
from concourse.bass_utils import run_bass_kernel_spmd

_orig_act_tables = hw_specs.get_activation_tables


def _patched_act_tables(arch):
    _E = mybir.ActivationFunctionType.Exp
    _L = mybir.ActivationFunctionType.Ln
    out = {}
    for name, fns in _orig_act_tables(arch).items():
        if name != "natural_log_exp_and_others":
            fns = fns - {_E, _L}
        out[name] = fns
    return out


bacc.get_activation_tables = _patched_act_tables

F32 = mybir.dt.float32
F32R = mybir.dt.float32r
BF16 = mybir.dt.bfloat16
F16 = mybir.dt.float16
USE_BF16 = True
MM_DT = BF16 if USE_BF16 else F32R
EXP = mybir.ActivationFunctionType.Exp
MUL = mybir.AluOpType.mult
ADD = mybir.AluOpType.add
SUB = mybir.AluOpType.subtract

B, S, D = 2, 2048, 1024
H, DK = 16, 64          # global heads, head dim
HL = 4                  # heads per core
GD = HL * DK            # local width 256
T = S // 512            # 4 q-tiles of 512
C = S // 128            # 16 kpos chunks of 128
DCH = D // 128          # 8 contraction chunks
THETA = 10000.0

_cache = {}


def _build_kernel():
    nc = bacc.Bacc("TRN2", target_bir_lowering=False, debug=False, num_devices=8)

    xT = nc.declare_dram_parameter("xT", [D, S], MM_DT, isOutput=False)
    wq = nc.declare_dram_parameter("wq", [D, GD], MM_DT, isOutput=False)
    wk = nc.declare_dram_parameter("wk", [D, GD], MM_DT, isOutput=False)
    wv = nc.declare_dram_parameter("wv", [D, GD], MM_DT, isOutput=False)
    wo = nc.declare_dram_parameter("wo", [GD, D], MM_DT, isOutput=False)
    ccd = nc.declare_dram_parameter("cc", [32, S], F32, isOutput=False)
    ssd = nc.declare_dram_parameter("ss", [32, S], F32, isOutput=False)
    mskd = nc.declare_dram_parameter("msk", [128, 2, 128], MM_DT,
                                     isOutput=False)
    out = nc.declare_dram_parameter("out", [S, D], F16, isOutput=True)

    with tile.TileContext(nc) as tc:
        with (
            tc.tile_pool(name="consts", bufs=1) as consts,
            tc.tile_pool(name="persist", bufs=1) as persist,
            tc.tile_pool(name="rtmp", bufs=8) as rtmp,
            tc.tile_pool(name="rop", bufs=3) as rop,
            tc.tile_pool(name="ep", bufs=8) as ep,
            tc.tile_pool(name="rp", bufs=6) as rp,
            tc.tile_pool(name="osb", bufs=3) as osb,
            tc.tile_pool(name="qp", bufs=2, space="PSUM") as qp,
            tc.tile_pool(name="scp", bufs=2, space="PSUM") as scp,
            tc.tile_pool(name="avp", bufs=2, space="PSUM") as avp,
        ):
            # ---- constants.  Emission order = DMA priority per queue.
            # sync queue: xt0/wq interleaved per d-chunk (first QK matmuls),
            # then wk, wv.  gpsimd queue: cc/ss/msk (needed by first RoPE /
            # first mask), later the qT repacks.  scalar queue: xt1-3, wo
            # (ACT idles until the first exp anyway).
            wq_t = consts.tile([128, DCH, GD], MM_DT, tag="wq")
            wk_t = consts.tile([128, DCH, GD], MM_DT, tag="wk")
            wv_t = consts.tile([128, DCH, GD], MM_DT, tag="wv")
            wo_t = consts.tile([128, 2, D], MM_DT, tag="wo")
            cc_t = consts.tile([128, S], F32, tag="cc")
            ss_t = consts.tile([128, S], F32, tag="ss")
            msk_t = consts.tile([128, 2, 128], MM_DT, tag="msk")

            xT_v = xT.rearrange("(c p) s -> p c s", p=128)
            wq_v = wq.rearrange("(c p) g -> p c g", p=128)

            xts = [consts.tile([128, DCH, 512], MM_DT, tag=f"xt{t}",
                               name=f"xt{t}") for t in range(T)]
            for dd in range(DCH):
                nc.sync.dma_start(out=xts[0][:, dd, :], in_=xT_v[:, dd, 0:512])
                nc.sync.dma_start(out=wq_t[:, dd, :], in_=wq_v[:, dd, :])
            nc.sync.dma_start(
                out=wk_t[:], in_=wk.rearrange("(c p) g -> p c g", p=128))
            nc.sync.dma_start(
                out=wv_t[:], in_=wv.rearrange("(c p) g -> p c g", p=128))
            # cc/ss: one DRAM read of rows 0:32, then log-replicate on-chip
            nc.gpsimd.dma_start(out=cc_t[0:32, :], in_=ccd[:])
            nc.gpsimd.dma_start(out=ss_t[0:32, :], in_=ssd[:])
            nc.gpsimd.dma_start(out=cc_t[32:64, :], in_=cc_t[0:32, :])
            nc.gpsimd.dma_start(out=ss_t[32:64, :], in_=ss_t[0:32, :])
            nc.gpsimd.dma_start(out=cc_t[64:128, :], in_=cc_t[0:64, :])
            nc.gpsimd.dma_start(out=ss_t[64:128, :], in_=ss_t[0:64, :])
            nc.gpsimd.dma_start(out=msk_t[:], in_=mskd[:])
            for t in range(1, T):
                for dd in range(DCH):
                    nc.sync.dma_start(
                        out=xts[t][:, dd, :],
                        in_=xT_v[:, dd, 512 * t:512 * (t + 1)])
            nc.sync.dma_start(
                out=wo_t[:], in_=wo.rearrange("(c p) d -> p c d", p=128))

            ones_f = consts.tile([128, 2, DK], F32, tag="onesf")
            nc.vector.memset(ones_f[:], 1.0)
            ones = consts.tile([128, 2, DK], MM_DT, tag="ones")
            nc.vector.tensor_copy(ones[:], ones_f[:])

            # persistent per-qtile activations (disjoint tiles so the
            # interleaved A(t+1) writes never alias B(t) reads)
            qTs = [persist.tile([128, 2, 512], MM_DT, tag=f"qT{t}",
                                name=f"qT{t}") for t in range(T)]
            kTs = [persist.tile([128, 2, 512], MM_DT, tag=f"kT{t}",
                                name=f"kT{t}") for t in range(T)]
            yTs = [persist.tile([128, 2, 512], MM_DT, tag=f"yT{t}",
                                name=f"yT{t}") for t in range(T)]
            vxs = [persist.tile([128, 4, HL, 2 * DK], MM_DT, tag=f"vx{t}",
                                name=f"vx{t}") for t in range(T)]

            # ones halves of v_ext: even heads [64:128], odd heads [0:64]
            for t in range(T):
                for s4 in range(4):
                    for par, sl in ((0, slice(DK, 2 * DK)),
                                    (1, slice(0, DK))):
                        nc.vector.tensor_copy(
                            vxs[t][:, s4, par::2, sl], ones[:])

            # ---- A unit generator: projections + RoPE + v packing for
            # qtile t, split into 8 emission units (4 QK + 4 V) ----
            def a_units(t):
                qs = slice(512 * t, 512 * (t + 1))
                xt = xts[t]

                def mk_qk(w_t, dst, dma_eng, oc, st, wn):
                    def emit():
                        ccs, sss = cc_t[:, qs], ss_t[:, qs]
                        ps = qp.tile([128, 512], F32, tag="qp",
                                     name=f"ps{t}_{oc}{wn}")
                        for d in range(DCH):
                            nc.tensor.matmul(
                                ps[:],
                                lhsT=w_t[:, d, 128 * oc:128 * (oc + 1)],
                                rhs=xt[:, d, :],
                                start=(d == 0),
                                stop=(d == DCH - 1),
                            )
                        if oc == 0:
                            st["t1"] = rtmp.tile([128, 512], F32, tag="rt",
                                                 name=f"t1_{t}{wn}")
                            st["t3"] = rtmp.tile([128, 512], F32, tag="rt",
                                                 name=f"t3_{t}{wn}")
                            st["ro"] = rop.tile([128, 2, 512], MM_DT,
                                                tag="ro", name=f"ro_{t}{wn}")
                            nc.vector.tensor_tensor(st["t1"][:], ps[:], ccs,
                                                    op=MUL)
                            nc.vector.tensor_tensor(st["t3"][:], ps[:], sss,
                                                    op=MUL)
                        else:
                            t2 = rtmp.tile([128, 512], F32, tag="rt",
                                           name=f"t2_{t}{wn}")
                            t4 = rtmp.tile([128, 512], F32, tag="rt",
                                           name=f"t4_{t}{wn}")
                            ro = st["ro"]
                            nc.vector.tensor_tensor(t2[:], ps[:], sss, op=MUL)
                            nc.vector.tensor_tensor(ro[:, 0, :], st["t1"][:],
                                                    t2[:], op=SUB)
                            nc.vector.tensor_tensor(t4[:], ps[:], ccs, op=MUL)
                            nc.vector.tensor_tensor(ro[:, 1, :], st["t3"][:],
                                                    t4[:], op=ADD)
                            # repack to per-head-contiguous rows
                            for half in range(2):
                                for oc2 in range(2):
                                    for hp in range(2):
                                        sp = 32 * (2 * oc2 + hp)
                                        dp = 64 * hp + 32 * half
                                        dma_eng.dma_start(
                                            out=dst[dp:dp + 32, oc2, :],
                                            in_=ro[sp:sp + 32, half, :],
                                        )
                    return emit

                for w_t, dst, dma_eng, wn in ((wq_t, qTs[t], nc.gpsimd, "q"),
                                              (wk_t, kTs[t], nc.sync, "k")):
                    st = {}
                    yield mk_qk(w_t, dst, dma_eng, 0, st, wn)
                    yield mk_qk(w_t, dst, dma_eng, 1, st, wn)

                def mk_v(s4):
                    def emit():
                        psv = qp.tile([128, 512], F32, tag="qp",
                                      name=f"psv{t}_{s4}")
                        for d in range(DCH):
                            nc.tensor.matmul(
                                psv[:, :GD],
                                lhsT=xt[:, d, 128 * s4:128 * (s4 + 1)],
                                rhs=wv_t[:, d, :],
                                start=(d == 0),
                                stop=(d == DCH - 1),
                            )
                        pv = psv[:, :GD].rearrange("p (h e) -> p h e", e=DK)
                        for par, sl in ((0, slice(0, DK)),
                                        (1, slice(DK, 2 * DK))):
                            nc.vector.tensor_copy(
                                vxs[t][:, s4, par::2, sl], pv[:, par::2, :])
                    return emit

                for s4 in range(4):
                    yield mk_v(s4)

            # ---- C unit generator: output projection for qtile t ----
            def c_units(t):
                def mk_c(s4, n):
                    def emit():
                        s_ = 4 * t + s4
                        nsl = slice(512 * n, 512 * (n + 1))
                        po = qp.tile([128, 512], F32, tag="qp",
                                     name=f"po{t}_{s4}_{n}")
                        for ldc in range(2):
                            nc.tensor.matmul(
                                po[:],
                                lhsT=yTs[t][:, ldc,
                                            128 * s4:128 * (s4 + 1)],
                                rhs=wo_t[:, ldc, nsl],
                                start=(ldc == 0),
                                stop=(ldc == 1),
                            )
                        ob = osb.tile([128, 512], F16, tag="ob")
                        if t == 3:
                            nc.scalar.copy(ob[:], po[:])
                        else:
                            nc.vector.tensor_copy(ob[:], po[:])
                        nc.sync.dma_start(
                            out=out[128 * s_:128 * (s_ + 1), nsl], in_=ob[:])
                    return emit

                for s4 in range(4):
                    for n in range(2):
                        yield mk_c(s4, n)

            # ---- main pipeline: A(0) prologue, then per qtile t run the
            # attention chunk loop with A(t+1)+C(t-1) units paced in as
            # PE filler; C(3) is the epilogue ----
            for em in a_units(0):
                em()

            for t in range(T):
                filler = []
                if t < T - 1:
                    filler.extend(a_units(t + 1))
                if t >= 1:
                    filler.extend(c_units(t - 1))
                nck = 4 * t + 4
                total_chunks = 2 * nck
                fq = len(filler)
                done_f = 0
                ci = 0

                def pump():
                    nonlocal done_f, ci
                    target = min(fq, (ci * fq) // total_chunks + 1)
                    while done_f < target:
                        filler[done_f]()
                        done_f += 1

                for pair in range(2):
                    heads = (2 * pair, 2 * pair + 1)
                    av_ps = {}
                    for h in heads:
                        av_ps[h] = avp.tile([128, 512], F32, tag="av",
                                            name=f"av_{t}_{h}")
                    # software-pipelined emission: AV runs two chunks
                    # behind QK/exp so the in-order PE queue never blocks
                    pend = []  # [(c, e, f0)] awaiting AV
                    for c in range(nck):
                        ks = slice(128 * c, 128 * (c + 1))
                        j = c - 4 * t
                        f0 = 128 * j if (0 < j < 4 and c > 0) else 0
                        kt_src = kTs[c // 4]
                        kks = slice(128 * (c % 4), 128 * (c % 4) + 128)
                        sc = scp.tile([128, 2, 512], F32, tag="sc")
                        for hp in range(2):
                            rows = slice(64 * hp, 64 * hp + 64)
                            nc.tensor.matmul(
                                sc[:, hp, f0:],
                                lhsT=kt_src[rows, pair, kks],
                                rhs=qTs[t][rows, pair, f0:],
                                start=True, stop=True,
                                tile_position=(64 * hp, 0),
                            )
                        e = ep.tile([128, 2, 512], MM_DT, tag="e")
                        nc.scalar.activation(e[:, :, f0:], sc[:, :, f0:], EXP)
                        if c >= 4 * t:  # diagonal tile: mask only the
                            # 128-col diagonal band [f0, f0+128)
                            nc.gpsimd.tensor_tensor(
                                e[:, :, f0:f0 + 128], e[:, :, f0:f0 + 128],
                                msk_t[:], op=MUL)
                        pend.append((c, e, f0))
                        if len(pend) > 2:
                            pc, pe_, pf0 = pend.pop(0)
                            for hp, h in enumerate(heads):
                                nc.tensor.matmul(
                                    av_ps[h][:, pf0:],
                                    lhsT=vxs[pc // 4][:, pc % 4, h, :],
                                    rhs=pe_[:, hp, pf0:],
                                    start=(pc == 0),
                                    stop=False,
                                )
                        ci += 1
                        pump()
                    while pend:
                        pc, pe_, pf0 = pend.pop(0)
                        for hp, h in enumerate(heads):
                            nc.tensor.matmul(
                                av_ps[h][:, pf0:],
                                lhsT=vxs[pc // 4][:, pc % 4, h, :],
                                rhs=pe_[:, hp, pf0:],
                                start=(pc == 0),
                                stop=(not pend),
                            )
                    for h in heads:
                        # sums rows / out rows by head parity
                        if h % 2 == 0:
                            srows, orows = slice(64, 128), slice(0, 64)
                        else:
                            srows, orows = slice(0, 64), slice(64, 128)
                        r2 = rp.tile([128, 512], F32, tag="rr")
                        # full-tile: the custom-DVE lowering mishandles
                        # base_partition=64; extra rows are free and unread
                        nc.vector.reciprocal_approx_fast(
                            out=r2[:], in_=av_ps[h][:])
                        nc.vector.tensor_tensor(
                            yTs[t][orows, h // 2, :],
                            av_ps[h][orows], r2[srows], op=MUL)
                # safety: drain any unfilled units for this t
                while done_f < fq:
                    filler[done_f]()
                    done_f += 1

            for em in c_units(3):
                em()

    nc.compile()
    return nc


def _host_prep(x, token_positions, Wq, Wk, Wv, Wo):
    # d_k permutation folded into Wq/Wk.  Projection-output row n (0..255):
    # chunk oc = n//128 (all x1 lanes in chunk 0, x2 in chunk 1 for RoPE),
    # head h = (n%128)//32, freq j = n%32 -> orig row 64h + 2j + oc.
    n = np.arange(GD)
    chunk = n // 128
    hh = (n % 128) // 32
    jj = n % 32
    perm = 64 * hh + 2 * jj + chunk

    pos = np.asarray(token_positions).astype(np.float64)
    inv_freq = THETA ** (-np.arange(0, DK, 2, dtype=np.float64) / DK)  # [32]
    ang = pos[:, None] * inv_freq[None, :]                             # [S, 32]
    cos = np.cos(ang).astype(np.float32)
    sin = np.sin(ang).astype(np.float32)
    cc = np.ascontiguousarray(cos.T)
    ss = np.ascontiguousarray(sin.T)

    # band mask for the 128-col diagonal band: msk[p, :, g] = 1.0 iff g >= p
    pp_, gg_ = np.arange(128)[:, None], np.arange(128)[None, :]
    msk = np.repeat((gg_ >= pp_)[:, None, :], 2, axis=1)  # [128, 2, 128]

    scale = 1.0 / np.sqrt(np.float32(DK))
    if USE_BF16:
        import ml_dtypes
        mmnp = ml_dtypes.bfloat16
    else:
        mmnp = np.float32
    in_maps = []
    for core in range(8):
        b, g = divmod(core, 4)
        gsl = slice(GD * g, GD * (g + 1))
        in_maps.append({
            "xT": np.ascontiguousarray(np.asarray(x[b], np.float32).T).astype(mmnp),
            "wq": np.ascontiguousarray(
                (np.asarray(Wq[gsl], np.float32) * scale)[perm].T.astype(mmnp)),
            "wk": np.ascontiguousarray(np.asarray(Wk[gsl], np.float32)[perm].T.astype(mmnp)),
            "wv": np.ascontiguousarray(np.asarray(Wv[gsl], np.float32).T.astype(mmnp)),
            "wo": np.ascontiguousarray(np.asarray(Wo[:, gsl], np.float32).T.astype(mmnp)),
            "cc": cc,
            "ss": ss,
            "msk": np.ascontiguousarray(msk).astype(mmnp),
        })
    return in_maps


def kernel(x, token_positions, Wq, Wk, Wv, Wo, _trace=False, _result=[None],
           _tmpdir=None):
    if "nc" not in _cache:
        _cache["nc"] = _build_kernel()
    nc = _cache["nc"]
    in_maps = _host_prep(x, token_positions, Wq, Wk, Wv, Wo)
    res = None
    for attempt in range(3):
        try:
            res = run_bass_kernel_spmd(
                nc, in_maps, core_ids=list(range(8)), trace=_trace,
                tmpdir=_tmpdir)
            break
        except Exception:
            # transient NRT_EXEC_UNIT_UNRECOVERABLE device hiccups resolve
            # on retry
            if attempt == 2:
                raise
    _result[0] = res
    outs = np.stack([np.asarray(r["out"], np.float32) for r in res.results])
    full = outs.reshape(B, 4, S, D).sum(axis=1, dtype=np.float32)
    return full


# revision 30
# speedup vs baseline: 1.0212x; 1.0212x over previous
"""Causal multihead self-attention with RoPE on 8 TRN2 NeuronCores.

Problem: B=2, S=2048, D=1024, H=16 heads, d_k=64, causal, RoPE theta=10000.

Sharding (Megatron-style): core c = 4*b + g handles batch b and the 4 heads
[4g, 4g+4): Wq/Wk/Wv column-parallel, Wo row-parallel; each core emits a
partial [S, D] fp16 output and the host sums the 4 partials per batch.

v3: software-pipelined across qtiles.  The attention phase (B) is
ACT-engine-bound (the softmax exp stream); the projection phase (A) and
output-projection phase (C) are PE-bound.  v3 interleaves the EMISSION of
A(t+1) and C(t-1) matmul units into B(t)'s chunk loop so the in-order PE
queue always has runnable work while ACT crunches exp — instead of the
phases running back-to-back with each engine idling in the other's phase.
Per-qtile activation tiles (qT/kT/yT/v_ext split by t) keep the interleaved
writes/reads on disjoint tiles so no false dependencies serialize the
pipeline.  Softmax 1/sums runs on DVE (reciprocal_approx_fast) instead of
ACT Ln+Exp; causal masking multiplies only the 128-col diagonal band (the
rest of a diagonal tile is fully masked -> skipped, or fully unmasked) and
runs on GpSimd; output partials are staged fp16.
"""
import sys

sys.path.insert(0, "/opt/trn_rl_repo")

import numpy as np

import concourse.bacc as bacc
import concourse.hw_specs as hw_specs
import concourse.tile as tile
from concourse import mybir
from concourse.bass_utils import run_bass_kernel_spmd

_orig_act_tables = hw_specs.get_activation_tables


def _patched_act_tables(arch):
    _E = mybir.ActivationFunctionType.Exp
    _L = mybir.ActivationFunctionType.Ln
    out = {}
    for name, fns in _orig_act_tables(arch).items():
        if name != "natural_log_exp_and_others":
            fns = fns - {_E, _L}
        out[name] = fns
    return out


bacc.get_activation_tables = _patched_act_tables

F32 = mybir.dt.float32
F32R = mybir.dt.float32r
BF16 = mybir.dt.bfloat16
F16 = mybir.dt.float16
USE_BF16 = True
MM_DT = BF16 if USE_BF16 else F32R
EXP = mybir.ActivationFunctionType.Exp
MUL = mybir.AluOpType.mult
ADD = mybir.AluOpType.add
SUB = mybir.AluOpType.subtract

B, S, D = 2, 2048, 1024
H, DK = 16, 64          # global heads, head dim
HL = 4                  # heads per core
GD = HL * DK            # local width 256
T = S // 512            # 4 q-tiles of 512
C = S // 128            # 16 kpos chunks of 128
DCH = D // 128          # 8 contraction chunks
THETA = 10000.0

_cache = {}


def _build_kernel():
    nc = bacc.Bacc("TRN2", target_bir_lowering=False, debug=False, num_devices=8)

    xT = nc.declare_dram_parameter("xT", [D, S], MM_DT, isOutput=False)
    wq = nc.declare_dram_parameter("wq", [D, GD], MM_DT, isOutput=False)
    wk = nc.declare_dram_parameter("wk", [D, GD], MM_DT, isOutput=False)
    wv = nc.declare_dram_parameter("wv", [D, GD], MM_DT, isOutput=False)
    wo = nc.declare_dram_parameter("wo", [GD, D], MM_DT, isOutput=False)
    ccd = nc.declare_dram_parameter("cc", [32, S], F32, isOutput=False)
    ssd = nc.declare_dram_parameter("ss", [32, S], F32, isOutput=False)
    mskd = nc.declare_dram_parameter("msk", [128, 2, 128], MM_DT,
                                     isOutput=False)
    out = nc.declare_dram_parameter("out", [S, D], F16, isOutput=True)

    with tile.TileContext(nc) as tc:
        with (
            tc.tile_pool(name="consts", bufs=1) as consts,
            tc.tile_pool(name="persist", bufs=1) as persist,
            tc.tile_pool(name="rtmp", bufs=8) as rtmp,
            tc.tile_pool(name="rop", bufs=3) as rop,
            tc.tile_pool(name="ep", bufs=8) as ep,
            tc.tile_pool(name="rp", bufs=6) as rp,
            tc.tile_pool(name="osb", bufs=3) as osb,
            tc.tile_pool(name="qp", bufs=2, space="PSUM") as qp,
            tc.tile_pool(name="scp", bufs=2, space="PSUM") as scp,
            tc.tile_pool(name="avp", bufs=2, space="PSUM") as avp,
        ):
            # ---- constants.  Emission order = DMA priority per queue.
            # sync queue: xt0/wq interleaved per d-chunk (first QK matmuls),
            # then wk, wv.  gpsimd queue: cc/ss/msk (needed by first RoPE /
            # first mask), later the qT repacks.  scalar queue: xt1-3, wo
            # (ACT idles until the first exp anyway).
            wq_t = consts.tile([128, DCH, GD], MM_DT, tag="wq")
            wk_t = consts.tile([128, DCH, GD], MM_DT, tag="wk")
            wv_t = consts.tile([128, DCH, GD], MM_DT, tag="wv")
            wo_t = consts.tile([128, 2, D], MM_DT, tag="wo")
            cc_t = consts.tile([128, S], F32, tag="cc")
            ss_t = consts.tile([128, S], F32, tag="ss")
            msk_t = consts.tile([128, 2, 128], MM_DT, tag="msk")

            xT_v = xT.rearrange("(c p) s -> p c s", p=128)
            wq_v = wq.rearrange("(c p) g -> p c g", p=128)

            # each dma_start costs ~600ns on the issuing queue engine, so
            # batch everything into whole-tile transfers and spread queues:
            # sync gets the A(0)-critical loads + kT repacks + out stores,
            # scalar gets the rest of x and wo, gpsimd gets cc/ss/msk + qT
            # repacks.
            xts = [consts.tile([128, DCH, 512], MM_DT, tag=f"xt{t}",
                               name=f"xt{t}") for t in range(T)]
            nc.sync.dma_start(out=xts[0][:], in_=xT_v[:, :, 0:512])
            nc.sync.dma_start(out=wq_t[:], in_=wq_v[:])
            nc.sync.dma_start(
                out=wk_t[:], in_=wk.rearrange("(c p) g -> p c g", p=128))
            nc.sync.dma_start(
                out=wv_t[:], in_=wv.rearrange("(c p) g -> p c g", p=128))
            # cc/ss: one DRAM read of rows 0:32, then log-replicate on-chip
            nc.gpsimd.dma_start(out=cc_t[0:32, :], in_=ccd[:])
            nc.gpsimd.dma_start(out=ss_t[0:32, :], in_=ssd[:])
            nc.gpsimd.dma_start(out=cc_t[32:64, :], in_=cc_t[0:32, :])
            nc.gpsimd.dma_start(out=ss_t[32:64, :], in_=ss_t[0:32, :])
            nc.gpsimd.dma_start(out=cc_t[64:128, :], in_=cc_t[0:64, :])
            nc.gpsimd.dma_start(out=ss_t[64:128, :], in_=ss_t[0:64, :])
            nc.gpsimd.dma_start(out=msk_t[:], in_=mskd[:])
            for t in range(1, T):
                nc.scalar.dma_start(
                    out=xts[t][:], in_=xT_v[:, :, 512 * t:512 * (t + 1)])
            nc.scalar.dma_start(
                out=wo_t[:], in_=wo.rearrange("(c p) d -> p c d", p=128))

            ones = consts.tile([128, 512], MM_DT, tag="ones")
            nc.vector.memset(ones[:], 1.0)
            ones_v = ones.rearrange("p (a b c) -> p a b c", a=4, b=2, c=DK)

            # persistent per-qtile activations (disjoint tiles so the
            # interleaved A(t+1) writes never alias B(t) reads)
            qTs = [persist.tile([128, 2, 512], MM_DT, tag=f"qT{t}",
                                name=f"qT{t}") for t in range(T)]
            kTs = [persist.tile([128, 2, 512], MM_DT, tag=f"kT{t}",
                                name=f"kT{t}") for t in range(T)]
            yTs = [persist.tile([128, 2, 512], MM_DT, tag=f"yT{t}",
                                name=f"yT{t}") for t in range(T)]
            vxs = [persist.tile([128, 4, HL, 2 * DK], MM_DT, tag=f"vx{t}",
                                name=f"vx{t}") for t in range(T)]

            def emit_vx_ones(t):
                # ones halves of v_ext: even heads [64:128], odd heads
                # [0:64]; one copy per parity covering all 4 s4 chunks
                for par, sl in ((0, slice(DK, 2 * DK)), (1, slice(0, DK))):
                    nc.vector.tensor_copy(vxs[t][:, :, par::2, sl],
                                          ones_v[:])

            emit_vx_ones(0)

            # ---- A unit generator: projections + RoPE + v packing for
            # qtile t, split into 8 emission units (4 QK + 4 V) ----
            def a_units(t):
                qs = slice(512 * t, 512 * (t + 1))
                xt = xts[t]

                def mk_qk(w_t, dst, dma_eng, oc, st, wn):
                    def emit():
                        ccs, sss = cc_t[:, qs], ss_t[:, qs]
                        ps = qp.tile([128, 512], F32, tag="qp",
                                     name=f"ps{t}_{oc}{wn}")
                        for d in range(DCH):
                            nc.tensor.matmul(
                                ps[:],
                                lhsT=w_t[:, d, 128 * oc:128 * (oc + 1)],
                                rhs=xt[:, d, :],
                                start=(d == 0),
                                stop=(d == DCH - 1),
                            )
                        if oc == 0:
                            st["t1"] = rtmp.tile([128, 512], F32, tag="rt",
                                                 name=f"t1_{t}{wn}")
                            st["t3"] = rtmp.tile([128, 512], F32, tag="rt",
                                                 name=f"t3_{t}{wn}")
                            st["ro"] = rop.tile([128, 2, 512], MM_DT,
                                                tag="ro", name=f"ro_{t}{wn}")
                            nc.vector.tensor_tensor(st["t1"][:], ps[:], ccs,
                                                    op=MUL)
                            nc.vector.tensor_tensor(st["t3"][:], ps[:], sss,
                                                    op=MUL)
                        else:
                            t2 = rtmp.tile([128, 512], F32, tag="rt",
                                           name=f"t2_{t}{wn}")
                            t4 = rtmp.tile([128, 512], F32, tag="rt",
                                           name=f"t4_{t}{wn}")
                            ro = st["ro"]
                            nc.vector.tensor_tensor(t2[:], ps[:], sss, op=MUL)
                            nc.vector.tensor_tensor(ro[:, 0, :], st["t1"][:],
                                                    t2[:], op=SUB)
                            nc.vector.tensor_tensor(t4[:], ps[:], ccs, op=MUL)
                            nc.vector.tensor_tensor(ro[:, 1, :], st["t3"][:],
                                                    t4[:], op=ADD)
                            # repack to per-head-contiguous rows:
                            # dst row 64*hp + 32*half + j, chunk oc2
                            # <- ro row 32*(2*oc2+hp) + j, half
                            for half in range(2):
                                for oc2 in range(2):
                                    for hp in range(2):
                                        sp = 32 * (2 * oc2 + hp)
                                        dp = 64 * hp + 32 * half
                                        dma_eng.dma_start(
                                            out=dst[dp:dp + 32, oc2, :],
                                            in_=ro[sp:sp + 32, half, :],
                                        )
                    return emit

                for w_t, dst, dma_eng, wn in ((wq_t, qTs[t], nc.gpsimd, "q"),
                                              (wk_t, kTs[t], nc.sync, "k")):
                    st = {}
                    yield mk_qk(w_t, dst, dma_eng, 0, st, wn)
                    yield mk_qk(w_t, dst, dma_eng, 1, st, wn)

                def mk_v(s4):
                    def emit():
                        if s4 == 0 and t > 0:
                            emit_vx_ones(t)
                        psv = qp.tile([128, 512], F32, tag="qp",
                                      name=f"psv{t}_{s4}")
                        for d in range(DCH):
                            nc.tensor.matmul(
                                psv[:, :GD],
                                lhsT=xt[:, d, 128 * s4:128 * (s4 + 1)],
                                rhs=wv_t[:, d, :],
                                start=(d == 0),
                                stop=(d == DCH - 1),
                            )
                        pv = psv[:, :GD].rearrange("p (h e) -> p h e", e=DK)
                        for par, sl in ((0, slice(0, DK)),
                                        (1, slice(DK, 2 * DK))):
                            nc.vector.tensor_copy(
                                vxs[t][:, s4, par::2, sl], pv[:, par::2, :])
                    return emit

                for s4 in range(4):
                    yield mk_v(s4)

            # ---- C unit generator: output projection for qtile t.
            # One unit per 128-row output block: 2 psum tiles, 2 staging
            # copies into a [128, 1024] fp16 tile, ONE store DMA ----
            def c_units(t):
                def mk_c(s4):
                    def emit():
                        s_ = 4 * t + s4
                        ob = osb.tile([128, 2, 512], F16, tag="ob")
                        for n in range(2):
                            po = qp.tile([128, 512], F32, tag="qp",
                                         name=f"po{t}_{s4}_{n}")
                            for ldc in range(2):
                                nc.tensor.matmul(
                                    po[:],
                                    lhsT=yTs[t][:, ldc,
                                                128 * s4:128 * (s4 + 1)],
                                    rhs=wo_t[:, ldc,
                                             512 * n:512 * (n + 1)],
                                    start=(ldc == 0),
                                    stop=(ldc == 1),
                                )
                            if t == 3:
                                nc.scalar.copy(ob[:, n, :], po[:])
                            else:
                                nc.vector.tensor_copy(ob[:, n, :], po[:])
                        nc.sync.dma_start(
                            out=out[128 * s_:128 * (s_ + 1), :], in_=ob[:])
                    return emit

                for s4 in range(4):
                    yield mk_c(s4)

            # ---- main pipeline: A(0) prologue, then per qtile t run the
            # attention chunk loop with A(t+1)+C(t-1) units paced in as
            # PE filler; C(3) is the epilogue ----
            for em in a_units(0):
                em()

            for t in range(T):
                filler = []
                if t < T - 1:
                    filler.extend(a_units(t + 1))
                if t >= 1:
                    filler.extend(c_units(t - 1))
                nck = 4 * t + 4
                total_chunks = 2 * nck
                fq = len(filler)
                done_f = 0
                ci = 0

                def pump():
                    nonlocal done_f, ci
                    target = min(fq, (ci * fq) // total_chunks + 1)
                    while done_f < target:
                        filler[done_f]()
                        done_f += 1

                for pair in range(2):
                    heads = (2 * pair, 2 * pair + 1)
                    av_ps = {}
                    for h in heads:
                        av_ps[h] = avp.tile([128, 512], F32, tag="av",
                                            name=f"av_{t}_{h}")
                    # software-pipelined emission: AV runs two chunks
                    # behind QK/exp so the in-order PE queue never blocks
                    pend = []  # [(c, e, f0)] awaiting AV
                    for c in range(nck):
                        ks = slice(128 * c, 128 * (c + 1))
                        j = c - 4 * t
                        f0 = 128 * j if (0 < j < 4 and c > 0) else 0
                        kt_src = kTs[c // 4]
                        kks = slice(128 * (c % 4), 128 * (c % 4) + 128)
                        sc = scp.tile([128, 2, 512], F32, tag="sc")
                        for hp in range(2):
                            rows = slice(64 * hp, 64 * hp + 64)
                            nc.tensor.matmul(
                                sc[:, hp, f0:],
                                lhsT=kt_src[rows, pair, kks],
                                rhs=qTs[t][rows, pair, f0:],
                                start=True, stop=True,
                                tile_position=(64 * hp, 0),
                            )
                        e = ep.tile([128, 2, 512], MM_DT, tag="e")
                        nc.scalar.activation(e[:, :, f0:], sc[:, :, f0:], EXP)
                        if c >= 4 * t:  # diagonal tile: mask only the
                            # 128-col diagonal band [f0, f0+128)
                            nc.gpsimd.tensor_tensor(
                                e[:, :, f0:f0 + 128], e[:, :, f0:f0 + 128],
                                msk_t[:], op=MUL)
                        pend.append((c, e, f0))
                        if len(pend) > 2:
                            pc, pe_, pf0 = pend.pop(0)
                            for hp, h in enumerate(heads):
                                nc.tensor.matmul(
                                    av_ps[h][:, pf0:],
                                    lhsT=vxs[pc // 4][:, pc % 4, h, :],
                                    rhs=pe_[:, hp, pf0:],
                                    start=(pc == 0),
                                    stop=False,
                                )
                        ci += 1
                        pump()
                    while pend:
                        pc, pe_, pf0 = pend.pop(0)
                        for hp, h in enumerate(heads):
                            nc.tensor.matmul(
                                av_ps[h][:, pf0:],
                                lhsT=vxs[pc // 4][:, pc % 4, h, :],
                                rhs=pe_[:, hp, pf0:],
                                start=(pc == 0),
                                stop=(not pend),
                            )
                    for h in heads:
                        # sums rows / out rows by head parity
                        if h % 2 == 0:
                            srows, orows = slice(64, 128), slice(0, 64)
                        else:
                            srows, orows = slice(0, 64), slice(64, 128)
                        r2 = rp.tile([128, 512], F32, tag="rr")
                        # full-tile: the custom-DVE lowering mishandles
                        # base_partition=64; extra rows are free and unread
                        nc.vector.reciprocal_approx_fast(
                            out=r2[:], in_=av_ps[h][:])
                        nc.vector.tensor_tensor(
                            yTs[t][orows, h // 2, :],
                            av_ps[h][orows], r2[srows], op=MUL)
                # safety: drain any unfilled units for this t
                while done_f < fq:
                    filler[done_f]()
                    done_f += 1

            for em in c_units(3):
                em()

    nc.compile()
    return nc


def _host_prep(x, token_positions, Wq, Wk, Wv, Wo):
    # d_k permutation folded into Wq/Wk.  Projection-output row n (0..255):
    # chunk oc = n//128 (all x1 lanes in chunk 0, x2 in chunk 1 for RoPE),
    # head h = (n%128)//32, freq j = n%32 -> orig row 64h + 2j + oc.
    n = np.arange(GD)
    chunk = n // 128
    hh = (n % 128) // 32
    jj = n % 32
    perm = 64 * hh + 2 * jj + chunk

    pos = np.asarray(token_positions).astype(np.float64)
    inv_freq = THETA ** (-np.arange(0, DK, 2, dtype=np.float64) / DK)  # [32]
    ang = pos[:, None] * inv_freq[None, :]                             # [S, 32]
    cos = np.cos(ang).astype(np.float32)
    sin = np.sin(ang).astype(np.float32)
    cc = np.ascontiguousarray(cos.T)
    ss = np.ascontiguousarray(sin.T)

    # band mask for the 128-col diagonal band: msk[p, :, g] = 1.0 iff g >= p
    pp_, gg_ = np.arange(128)[:, None], np.arange(128)[None, :]
    msk = np.repeat((gg_ >= pp_)[:, None, :], 2, axis=1)  # [128, 2, 128]

    scale = 1.0 / np.sqrt(np.float32(DK))
    if USE_BF16:
        import ml_dtypes
        mmnp = ml_dtypes.bfloat16
    else:
        mmnp = np.float32
    in_maps = []
    for core in range(8):
        b, g = divmod(core, 4)
        gsl = slice(GD * g, GD * (g + 1))
        in_maps.append({
            "xT": np.ascontiguousarray(np.asarray(x[b], np.float32).T).astype(mmnp),
            "wq": np.ascontiguousarray(
                (np.asarray(Wq[gsl], np.float32) * scale)[perm].T.astype(mmnp)),
            "wk": np.ascontiguousarray(np.asarray(Wk[gsl], np.float32)[perm].T.astype(mmnp)),
            "wv": np.ascontiguousarray(np.asarray(Wv[gsl], np.float32).T.astype(mmnp)),
            "wo": np.ascontiguousarray(np.asarray(Wo[:, gsl], np.float32).T.astype(mmnp)),
            "cc": cc,
            "ss": ss,
            "msk": np.ascontiguousarray(msk).astype(mmnp),
        })
    return in_maps


def kernel(x, token_positions, Wq, Wk, Wv, Wo, _trace=False, _result=[None],
           _tmpdir=None):
    if "nc" not in _cache:
        _cache["nc"] = _build_kernel()
    nc = _cache["nc"]
    in_maps = _host_prep(x, token_positions, Wq, Wk, Wv, Wo)
    res = None
    for attempt in range(3):
        try:
            res = run_bass_kernel_spmd(
                nc, in_maps, core_ids=list(range(8)), trace=_trace,
                tmpdir=_tmpdir)
            break
        except Exception:
            # transient NRT_EXEC_UNIT_UNRECOVERABLE device hiccups resolve
            # on retry
            if attempt == 2:
                raise
    _result[0] = res
    outs = np.stack([np.asarray(r["out"], np.float32) for r in res.results])
    full = outs.reshape(B, 4, S, D).sum(axis=1, dtype=np.float32)
    return full
